# revision 53
# baseline (speedup 1.0000x reference)
"""GuidedFilter Trainium2 kernel: batch-parallel over 8 NeuronCores.

Per core: img [1,512,512], feat [16,512,512] -> out [16,512,512] f32.
Host pre-casts feat/img to bf16 (same rounding the SWDGE cast applied
on-device before), so all loads ride HWDGE and Pool keeps no DMA duty.

Each 2-D reflect box blur (radius 5) is two TensorE passes, BOTH in
data-as-weights form with tight band windows:
  pass A: T1[w, i] = sum_h X[h, w] * B[i, h]   (lhsT = X column block,
          rhs = G window; psum partitions = w, free = i)
  pass C: out[i, u] = sum_w T1[w, i] * B[u, w] (lhsT = T1 column block,
          rhs = same G; psum partitions = i, free = u)
Layout is preserved: [h-part within 128-block, free=(block, w)] in and out.
Each source block j only touches output rows [128j-5, 128j+132] (reflect at
edges), so each pass streams 542 rows instead of 2048: band-overlap strips
(10 cols at block seams) are written as separate accumulate matmuls.
G is the unnormalized box matrix (entries {0,1,2}, exact in bf16); the
1/121 normalization is folded into the psum-evacuation ops.

Both elementwise subtractions are folded into TensorE via blur linearity,
with the b-side rewritten to decouple its inputs from the a-chain:
  a = t2 - t1            -> blur(a) = dual A-pass of (t2,+G16),(t1,-G16)
  b = mp - a*mI = mp*q - t2*mI  (q = 1 + mIR*mI, shared)
                         -> blur(b) = dual A-pass of (z1,+G16),(z2,-G16)
so a, u2, b never materialize. Dual-blur inputs (t1, t2, z1, z2) are fp16
(same matmul/DVE speed as bf16, 8x finer mantissa — keeps the z1-z2
cancellation benign).

Engine balance (Pool 90% / DVE 86% / Act 84% / PE 76% busy; Pool's
multiply stream paces the steady state):
  Act: T1 psum->sbuf handoffs (halves, [1024] each) + mp evac.
  DVE: psum stt consumers (t2/v/o) + z2 mul + z1 upper half.
  Pool: t1 and Pd muls + z1 lower half (gpsimd is 2.4x slower per
  element but otherwise idle).
  Act and DVE work per channel differ by less than one [1024] evac
  quantum, so the mp evac's first half is split into Act[512]+DVE[512]
  quarter-ops to equalize them (MP_ENG=("S","A")); the last channel
  reverts to plain Act since the tail is DVE-bound (MP_AA_CHANS).
  Img-stage blurs evacuate via a mixed DVE/Act half assignment
  (IMG_HOFF/MIS_ENG) found by joint sweep — each alone regresses.
  Output is stored bf16 in two row-halves (first half overlaps the second
  o-stt; host upcasts to f32), trimming the drain tail.

Blur2 calls are emitted software-pipelined: the C-pass of blur k is emitted
after the A-pass of blur k+1 (PENDING=1), and the per-channel a/b blurs
trail the X/P blurs by LEAD channels, hiding psum handoff latency from the
in-order PE queue. Pd for channel d+2 is emitted at iteration d.

An fp8e4 DoubleRow path for the X-blur A-pass (X_FP8, default off) is
implemented and walrus-verified: it cuts PE busy by 7us as predicted but
loses 2.3us net — the extra fp8 feat stream on the single shared DMA
device outweighs it since PE is not the binding engine.
"""
import sys

sys.path.insert(0, "/opt/trn_rl_repo")

import numpy as np
import ml_dtypes

RADIUS = 5
EPS = 1e-08
H = W = 512
D = 16
NCORES = 8
U = 1.0 / 121.0  # box normalization (11x11)

# ---- schedule / engine-assignment knobs (sweepable via cfg) -------------
DEFAULT_CFG = dict(
    LEAD=3,
    PENDING=1,
    TAPER=False,
    IMG_HOFF=("V", "A"),
    MIS_ENG=("A", "V"),
    MP_ENG=("S", "A"),
    MP_SX=512,        # Act share of an "S"-split mp evac half
    DEFER_MP=False,   # emit mp half 1 late (fills Act stall at dual blurs)
    INTERLEAVE=False,  # X, a, P, b emission order instead of X, P, a, b
    PRE_XP0=False,    # emit X0 blur before img blurs (earlier first evac)
    CFULL=False,      # single [128,2048] C psum tile, one-op consumers
    STORE_HALVES=True,  # store out rows 0-255 as soon as o half 0 is done
    OUT_BF16=True,    # store out as bf16 (host upcasts); halves store DMA
    T2_ENG=("V", "V"),
    V_ENG=("V", "V"),
    O_ENG=("V", "V"),
    T1_ENG="P",
    PD_ENG="P",
    Z1_SPLIT=True,    # z1 lower slice [0:Z1_X] on Pool, rest on DVE
    Z1_X=1024,        # split point of z1 between Pool and DVE
    Z2_SPLIT=False,   # z2 lower half on Pool, upper on DVE
    LOAD0_SINGLES=True,  # first feat pair as two single-channel loads
    MP_AA_CHANS=(15,),  # channels whose mp evac is plain ("A","A")
    Z1_POOL_CHANS=(),  # channels whose z1 runs fully on Pool
    LOAD_I_FIRST=True,  # img load issued before G/Gn
    DEFER_GN=True,    # Gn load/convert deferred past first feat loads
    IMG_POOL=False,   # I2/m2/mIR/q on Pool instead of DVE
    O_PE_CHANS=(),    # channels whose o runs as PE accum(121*v) + Act evac
    XMID=False,       # emission order I, X0, I2, P0 at start
    Z2_POOL_CHANS=(14, 15),  # channels whose z2 runs fully on Pool
    IMG_OPT=False,    # m2/I2 via Act Square, EPS via Act bias-copy
    PD_PAIR=False,    # Pd for channel pairs in one [128,4096] Pool op
    T1_X=2048,        # t1 split point: [0:x] Pool, rest DVE
    PD_X=2048,        # Pd split point: [0:x] Pool, rest DVE
    I_CHUNKS=False,   # img load + I2 as 4 j-chunks (earlier first A-pass)
    X_FP8=False,      # X-blur A-pass as fp8e4 DoubleRow (2x PE on that pass)
    # which T1 evac halves go to DVE, per channel: dict d -> list of
    # (blur_key, half) with blur_key in {X,P,a,b}, half in {0,1}
    SPLIT={},
)

# Per source block j: output-row window [lo, hi) touched by its 128 rows.
WIN = [(0, 133), (123, 261), (251, 389), (379, 512)]
GOFF = [0, 133, 271, 409]  # column offset of window j in packed G
GW = 542
# Segments per source block: (out_lo, out_hi, start, stop).
SEGS = [
    [(0, 123, True, True), (123, 133, True, False)],
    [(123, 133, False, True), (133, 251, True, True), (251, 261, True, False)],
    [(251, 261, False, True), (261, 379, True, True), (379, 389, True, False)],
    [(379, 389, False, True), (389, 512, True, True)],
]


def _box_matrix():
    B = np.zeros((512, 512), np.float32)
    for i in range(512):
        for d in range(-RADIUS, RADIUS + 1):
            j = i + d
            if j < 0:
                j = -j
            elif j > 511:
                j = 1022 - j
            B[i, j] += 1.0
    return B


def _g_packed():
    """G [128, 542]: G[p, GOFF[j]+c] = B[WIN[j][0]+c, 128j+p]."""
    B = _box_matrix()
    cols = []
    for j in range(4):
        lo, hi = WIN[j]
        cols.append(B[lo:hi, 128 * j:128 * (j + 1)].T)
    return np.ascontiguousarray(np.concatenate(cols, axis=1)).astype(
        ml_dtypes.bfloat16)


def _build_bass(cfg=None):
    import concourse.bass as bass
    import concourse.bacc as bacc
    import concourse.tile as tile
    from concourse import mybir

    C = dict(DEFAULT_CFG)
    if cfg:
        C.update(cfg)
    LEAD, PENDING, TAPER = C["LEAD"], C["PENDING"], C["TAPER"]

    f32 = mybir.dt.float32
    bf16 = mybir.dt.bfloat16
    fp16 = mybir.dt.float16
    Alu = mybir.AluOpType
    Act = mybir.ActivationFunctionType

    nc = bacc.Bacc("TRN2", target_bir_lowering=False, debug=False,
                   num_devices=NCORES)

    feat_d = nc.dram_tensor("feat", [D, H, W], bf16, kind="ExternalInput").ap()
    img_d = nc.dram_tensor("img", [1, H, W], bf16, kind="ExternalInput").ap()
    g_d = nc.dram_tensor("gmat", [128, GW], bf16, kind="ExternalInput").ap()
    gn_d = nc.dram_tensor("gneg", [128, GW], bf16, kind="ExternalInput").ap()
    idm_d = nc.dram_tensor("idm", [128, 128], bf16, kind="ExternalInput").ap()
    fp8 = mybir.dt.float8e4
    feat8_d = nc.dram_tensor("feat8", [D, H, W], fp8,
                             kind="ExternalInput").ap()
    g0_d = nc.dram_tensor("g0", [128, 2 * GW], fp8,
                          kind="ExternalInput").ap()
    g1_d = nc.dram_tensor("g1", [128, 2 * GW], fp8,
                          kind="ExternalInput").ap()
    out_dt = bf16 if C["OUT_BF16"] else f32
    out_d = nc.dram_tensor("out", [D, H, W], out_dt,
                           kind="ExternalOutput").ap()

    def ld(dst, src2d):
        # HBM [512,512] bf16 -> SBUF [128, (j,w)] bf16 (HWDGE)
        nc.sync.dma_start(
            out=dst.rearrange("p (j w) -> p j w", j=4),
            in_=src2d.rearrange("(j p) w -> p j w", p=128))

    with tile.TileContext(nc) as tc:
        with (
            tc.tile_pool(name="consts", bufs=1) as consts,
            tc.tile_pool(name="shared", bufs=1) as shared,
            tc.tile_pool(name="chan", bufs=2) as chan,
            tc.tile_pool(name="psum", bufs=1, space="PSUM") as psum,
        ):
            I = consts.tile([128, 2048], bf16)
            if C["LOAD_I_FIRST"] and not C["PRE_XP0"]:
                if C["I_CHUNKS"]:
                    for j in range(4):
                        nc.sync.dma_start(
                            out=I[:, 512 * j:512 * (j + 1)],
                            in_=img_d[0][128 * j:128 * (j + 1), :])
                else:
                    ld(I, img_d[0])
            G = consts.tile([128, GW], bf16)
            nc.sync.dma_start(out=G[:], in_=g_d)
            if C["X_FP8"]:
                fp8 = mybir.dt.float8e4
                G08 = consts.tile([128, 2 * GW], fp8)
                nc.sync.dma_start(out=G08[:], in_=g0_d)
                G18 = consts.tile([128, 2 * GW], fp8)
                nc.sync.dma_start(out=G18[:], in_=g1_d)
            Gn = consts.tile([128, GW], bf16)
            if not C["DEFER_GN"]:
                nc.sync.dma_start(out=Gn[:], in_=gn_d)
            G16 = consts.tile([128, GW], fp16)
            nc.vector.tensor_copy(G16[:], G[:])
            Gn16 = consts.tile([128, GW], fp16)

            Idm = consts.tile([128, 128], bf16)

            def emit_gn():
                if C["DEFER_GN"]:
                    nc.sync.dma_start(out=Gn[:], in_=gn_d)
                nc.vector.tensor_copy(Gn16[:], Gn[:])
                if C["O_PE_CHANS"]:
                    nc.sync.dma_start(out=Idm[:], in_=idm_d)

            if not C["DEFER_GN"]:
                emit_gn()
            if not C["LOAD_I_FIRST"] and not C["PRE_XP0"]:
                ld(I, img_d[0])

            def copy_half(eng, dst, src, scale=None):
                if eng == "A":
                    if scale is None:
                        nc.scalar.copy(dst, src)
                    else:
                        nc.scalar.activation(dst, src, Act.Copy, 0.0, scale)
                elif eng == "V":
                    if scale is None:
                        nc.vector.tensor_copy(dst, src)
                    else:
                        nc.vector.tensor_scalar_mul(dst, src, scale)
                else:
                    if scale is None:
                        nc.gpsimd.tensor_copy(dst, src)
                    else:
                        nc.gpsimd.tensor_scalar_mul(dst, src, scale)

            def stt(eng, dst, ps, s, t, op0, op1):
                e = nc.vector if eng == "V" else nc.gpsimd
                e.scalar_tensor_tensor(dst, ps, s, t, op0=op0, op1=op1)

            def emit_pass(ph, inputs):
                """ph: [tileA(1024), tileB(1024)]; out tile t -> ph[t//2].
                inputs: list of (lhsT_fn, Gtile) accumulated into the same
                psum regions (linear combination folded into TensorE)."""
                last = len(inputs) - 1
                for t in range(4):
                    pst, base = ph[t // 2], 512 * (t % 2)
                    for j in range(4):
                        lo0 = WIN[j][0]
                        for (lo, hi, st, sp) in SEGS[j]:
                            for idx, (lhsT_fn, Gt) in enumerate(inputs):
                                nc.tensor.matmul(
                                    pst[:, base + lo:base + hi],
                                    lhsT_fn(t, j),
                                    Gt[:, GOFF[j] + lo - lo0:
                                       GOFF[j] + hi - lo0],
                                    start=(st if idx == 0 else False),
                                    stop=(sp if idx == last else False),
                                    skip_group_check=True)

            # ---- software-pipelined blur emission --------------------------
            pending = []  # deferred C-passes

            def emit_blur_A_dr(X8, hoff, consume):
                # X-blur A-pass via fp8 DoubleRow: contract 2 w-block planes,
                # the unwanted plane killed by a zero G plane. wb<3 pairs
                # (wb, wb+1) with G08=[G|0]; wb=3 pairs (2, 3) with G18=[0|G].
                DR = mybir.MatmulPerfMode.DoubleRow
                A01 = psum.tile([128, 1024], f32, tag="A01")
                A23 = psum.tile([128, 1024], f32, tag="A23")
                ph = [A01, A23]
                for t in range(4):
                    pst, base = ph[t // 2], 512 * (t % 2)
                    if t < 3:
                        x_lo = lambda j: 512 * j + 128 * t
                        Gt = G08
                    else:
                        x_lo = lambda j: 512 * j + 256
                        Gt = G18
                    for j in range(4):
                        lo0 = WIN[j][0]
                        for (lo, hi, st, sp) in SEGS[j]:
                            c0 = GOFF[j] + lo - lo0
                            c1 = GOFF[j] + hi - lo0
                            nc.tensor.matmul(
                                pst[:, base + lo:base + hi],
                                X8[:, x_lo(j):x_lo(j) + 256].rearrange(
                                    "k (two m) -> k two m", two=2),
                                Gt.rearrange("k (two n) -> k two n",
                                             two=2)[:, :, c0:c1],
                                start=st, stop=sp,
                                perf_mode=DR,
                                skip_group_check=True)
                T1 = chan.tile([128, 2048], bf16, tag="T1", bufs=4)
                for h, (src, lo) in enumerate(((A01, 0), (A23, 1024))):
                    if hoff[h] == "S":
                        copy_half("A", T1[:, lo:lo + 512], src[:, 0:512])
                        copy_half("V", T1[:, lo + 512:lo + 1024],
                                  src[:, 512:1024])
                    else:
                        copy_half(hoff[h], T1[:, lo:lo + 1024], src[:])
                pending.append((T1, consume))

            def emit_blur_A(Xs, hoff, consume):
                # Xs: list of (tile, Gtile) accumulated as sum_k G_k-blur(X_k)
                A01 = psum.tile([128, 1024], f32, tag="A01")
                A23 = psum.tile([128, 1024], f32, tag="A23")
                emit_pass([A01, A23], [
                    ((lambda Xk: (lambda wb, j: Xk[
                        :, 512 * j + 128 * wb:512 * j + 128 * (wb + 1)]))(Xk),
                     Gk) for (Xk, Gk) in Xs])
                T1 = chan.tile([128, 2048], bf16, tag="T1", bufs=4)
                for h, (src, lo) in enumerate(((A01, 0), (A23, 1024))):
                    if hoff[h] == "S":
                        copy_half("A", T1[:, lo:lo + 512], src[:, 0:512])
                        copy_half("V", T1[:, lo + 512:lo + 1024],
                                  src[:, 512:1024])
                    else:
                        copy_half(hoff[h], T1[:, lo:lo + 1024], src[:])
                pending.append((T1, consume))

            def flush_C():
                if not pending:
                    return
                T1, consume = pending.pop(0)
                if C["CFULL"]:
                    Cf = psum.tile([128, 2048], f32, tag="C")
                    C01, C23 = Cf[:, 0:1024], Cf[:, 1024:2048]
                    ph = [C01, C23]
                else:
                    C01 = psum.tile([128, 1024], f32, tag="C01")
                    C23 = psum.tile([128, 1024], f32, tag="C23")
                    ph = [C01, C23]
                emit_pass(ph, [(lambda ib, wb: T1[
                    :, 512 * wb + 128 * ib:512 * wb + 128 * (ib + 1)], G)])
                if C["CFULL"]:
                    consume(Cf[:, :], None)
                else:
                    consume(C01, C23)

            def blur(X, hoff, consume):
                if not isinstance(X, list):
                    X = [(X, G)]
                emit_blur_A(X, hoff, consume)
                if len(pending) > PENDING:
                    flush_C()

            def hoff_for(d, key):
                eng = ["A", "A"]
                for spec in C["SPLIT"].get(d, []):
                    k, h = spec[0], spec[1]
                    e = spec[2] if len(spec) > 2 else "V"
                    if k == key:
                        eng[h] = e
                return tuple(eng)

            # ---- shared (img) stage ---------------------------------------
            I2 = shared.tile([128, 2048], bf16)
            q = shared.tile([128, 2048], fp16)
            mIs = shared.tile([128, 2048], bf16)
            mIR = shared.tile([128, 2048], bf16)
            R = shared.tile([128, 2048], f32)
            m2 = shared.tile([128, 2048], f32)
            vps = shared.tile([128, 2048], f32)

            img_mul = (nc.gpsimd.tensor_mul if C["IMG_POOL"]
                       else nc.vector.tensor_mul)
            if not C["PRE_XP0"]:
                if C["IMG_OPT"]:
                    nc.scalar.activation(I2[:], I[:], Act.Square, 0.0, 1.0)
                elif C["I_CHUNKS"]:
                    for j in range(4):
                        sl = slice(512 * j, 512 * (j + 1))
                        nc.vector.tensor_mul(I2[:, sl], I[:, sl], I[:, sl])
                else:
                    img_mul(I2[:], I[:], I[:])

            def consume_I(C01, C23):
                if C["IMG_OPT"] and C23 is not None:
                    # m2 = (U*C_I)^2 straight from psum (Act), ahead of the
                    # mIs evac so the vps chain is not gated on mIs
                    nc.scalar.activation(m2[:, 0:1024], C01[:], Act.Square,
                                         0.0, U)
                    nc.scalar.activation(m2[:, 1024:2048], C23[:], Act.Square,
                                         0.0, U)
                    copy_half(C["MIS_ENG"][0], mIs[:, 0:1024], C01[:],
                              scale=U)
                    copy_half(C["MIS_ENG"][1], mIs[:, 1024:2048], C23[:],
                              scale=U)
                    return
                if C23 is None:
                    copy_half(C["MIS_ENG"][0], mIs[:], C01, scale=U)
                else:
                    copy_half(C["MIS_ENG"][0], mIs[:, 0:1024], C01[:],
                              scale=U)
                    copy_half(C["MIS_ENG"][1], mIs[:, 1024:2048], C23[:],
                              scale=U)
                img_mul(m2[:], mIs[:], mIs[:])

            def consume_I2(C01, C23):
                # vps = U*corrI_raw - mI^2 + EPS ; R = 1/vps ; mIR = mI*R
                if C23 is None:
                    stt("V", vps[:], C01, U, m2[:], Alu.mult, Alu.subtract)
                else:
                    stt("V", vps[:, 0:1024], C01[:], U, m2[:, 0:1024],
                        Alu.mult, Alu.subtract)
                    stt("V", vps[:, 1024:2048], C23[:], U, m2[:, 1024:2048],
                        Alu.mult, Alu.subtract)
                nc.vector.tensor_scalar_add(vps[:], vps[:], EPS)
                nc.vector.reciprocal_approx_fast(R[:], vps[:])
                img_mul(mIR[:], mIs[:], R[:])
                img_mul(q[:], mIR[:], mIs[:])
                if C["IMG_POOL"]:
                    nc.gpsimd.tensor_scalar_add(q[:], q[:], 1.0)
                else:
                    nc.vector.tensor_scalar_add(q[:], q[:], 1.0)

            if not C["PRE_XP0"] and not C["XMID"]:
                blur(I, C["IMG_HOFF"], consume_I)
                blur(I2, C["IMG_HOFF"], consume_I2)

            # ---- per-channel state ----------------------------------------
            ch_state = {}

            def load_pair(d0, singles=False):
                X8 = None
                X2 = chan.tile([128, 4096], bf16, tag="xd", bufs=3)
                if singles:
                    for k in range(2):
                        nc.sync.dma_start(
                            out=X2[:, 2048 * k:2048 * (k + 1)].rearrange(
                                "p (j w) -> p j w", j=4),
                            in_=feat_d[d0 + k].rearrange(
                                "(j p) w -> p j w", p=128))
                else:
                    nc.sync.dma_start(
                        out=X2.rearrange("p (c j w) -> p c j w", c=2, j=4),
                        in_=feat_d[d0:d0 + 2].rearrange(
                            "c (j p) w -> p c j w", p=128))
                if C["X_FP8"] and X8 is None:
                    fp8 = mybir.dt.float8e4
                    X8 = chan.tile([128, 4096], fp8, tag="xd8", bufs=3)
                    nc.sync.dma_start(
                        out=X8.rearrange("p (c j w) -> p c j w", c=2, j=4),
                        in_=feat8_d[d0:d0 + 2].rearrange(
                            "c (j p) w -> p c j w", p=128))
                for k in range(2):
                    ch_state[d0 + k] = {"Xd": X2[:, 2048 * k:2048 * (k + 1)],
                                        "X2": X2}
                    if C["X_FP8"]:
                        ch_state[d0 + k]["Xd8"] = X8[
                            :, 2048 * k:2048 * (k + 1)]

            II = None
            if C["PD_PAIR"]:
                II = consts.tile([128, 4096], bf16)
                nc.gpsimd.tensor_copy(II[:, 0:2048], I[:])
                nc.gpsimd.tensor_copy(II[:, 2048:4096], I[:])

            def make_Pd(d):
                # emitted well ahead so the engine queue never stalls on it
                pdx = C["PD_X"]
                if C["PD_ENG"] == "P" and pdx < 2048 and not C["PD_PAIR"]:
                    s = ch_state[d]
                    Pd = chan.tile([128, 2048], bf16, tag="pd", bufs=2)
                    nc.gpsimd.tensor_mul(Pd[:, 0:pdx], s["Xd"][:, 0:pdx],
                                         I[:, 0:pdx])
                    nc.vector.tensor_mul(Pd[:, pdx:2048], s["Xd"][:, pdx:2048],
                                         I[:, pdx:2048])
                    s["Pd"] = Pd
                    return
                if C["PD_PAIR"]:
                    d0 = d - d % 2
                    if "Pd" in ch_state[d]:
                        return
                    if d0 + 1 in ch_state:
                        # both channels of the pair in one Pool op
                        P2 = chan.tile([128, 4096], bf16, tag="pd", bufs=2)
                        nc.gpsimd.tensor_mul(P2[:], ch_state[d0]["X2"][:],
                                             II[:])
                        ch_state[d0]["Pd"] = P2[:, 0:2048]
                        ch_state[d0 + 1]["Pd"] = P2[:, 2048:4096]
                        return
                s = ch_state[d]
                Pd = chan.tile([128, 2048], bf16, tag="pd", bufs=2)
                if C["PD_ENG"] == "V":
                    nc.vector.tensor_mul(Pd[:], s["Xd"], I[:])
                else:
                    nc.gpsimd.tensor_mul(Pd[:], s["Xd"], I[:])
                s["Pd"] = Pd

            def emit_XP_head(d):
                s = ch_state[d]
                mp = chan.tile([128, 2048], bf16, tag="mp", bufs=3)
                s["mp"] = mp

                def consume_X(C01, C23):
                    t1 = chan.tile([128, 2048], fp16, tag="t1m", bufs=4)
                    z1 = chan.tile([128, 2048], fp16, tag="z1", bufs=4)
                    t1x = C["T1_X"]
                    def tmul(dst, a_, b_):
                        if C["T1_ENG"] == "V":
                            nc.vector.tensor_mul(dst, a_, b_)
                        elif t1x < 2048 and dst.shape[-1] == 2048:
                            nc.gpsimd.tensor_mul(dst[:, 0:t1x], a_[:, 0:t1x],
                                                 b_[:, 0:t1x])
                            nc.vector.tensor_mul(dst[:, t1x:2048],
                                                 a_[:, t1x:2048],
                                                 b_[:, t1x:2048])
                        else:
                            nc.gpsimd.tensor_mul(dst, a_, b_)

                    def half(h, Ch):
                        lo, hi = 1024 * h, 1024 * (h + 1)
                        copy_half(C["MP_ENG"][h], mp[:, lo:hi], Ch[:],
                                  scale=U)
                        tmul(t1[:, lo:hi], mp[:, lo:hi], mIR[:, lo:hi])
                        nc.vector.tensor_mul(z1[:, lo:hi], mp[:, lo:hi],
                                             q[:, lo:hi])

                    if C23 is None:
                        copy_half(C["MP_ENG"][0], mp[:], C01, scale=U)
                        tmul(t1[:], mp[:], mIR[:])
                        nc.vector.tensor_mul(z1[:], mp[:], q[:])
                    elif C["DEFER_MP"]:
                        half(0, C01)
                        s["mp1"] = lambda: half(1, C23)
                    else:
                        mp_eng = (("A", "A") if d in C["MP_AA_CHANS"]
                                  else C["MP_ENG"])
                        for h, Ch in ((0, C01), (1, C23)):
                            lo = 1024 * h
                            if mp_eng[h] == "S":
                                mx = C["MP_SX"]
                                copy_half("A", mp[:, lo:lo + mx],
                                          Ch[:, 0:mx], scale=U)
                                copy_half("V", mp[:, lo + mx:lo + 1024],
                                          Ch[:, mx:1024], scale=U)
                            elif mp_eng[h] == "Z":
                                copy_half("V", mp[:, lo:lo + 512],
                                          Ch[:, 0:512], scale=U)
                                copy_half("A", mp[:, lo + 512:lo + 1024],
                                          Ch[:, 512:1024], scale=U)
                            else:
                                copy_half(mp_eng[h], mp[:, lo:lo + 1024],
                                          Ch[:], scale=U)
                        tmul(t1[:], mp[:], mIR[:])
                        if d in C["Z1_POOL_CHANS"]:
                            nc.gpsimd.tensor_mul(z1[:], mp[:], q[:])
                        elif C["Z1_SPLIT"]:
                            xs = C["Z1_X"]
                            nc.gpsimd.tensor_mul(z1[:, 0:xs], mp[:, 0:xs],
                                                 q[:, 0:xs])
                            nc.vector.tensor_mul(z1[:, xs:2048],
                                                 mp[:, xs:2048],
                                                 q[:, xs:2048])
                        else:
                            nc.vector.tensor_mul(z1[:], mp[:], q[:])
                    s["t1"] = t1
                    s["z1"] = z1

                def consume_P(C01, C23):
                    t2 = chan.tile([128, 2048], fp16, tag="t2", bufs=4)
                    if C23 is None:
                        stt(C["T2_ENG"][0], t2[:], C01, U, R[:],
                            Alu.mult, Alu.mult)
                    else:
                        stt(C["T2_ENG"][0], t2[:, 0:1024], C01[:], U,
                            R[:, 0:1024], Alu.mult, Alu.mult)
                        stt(C["T2_ENG"][1], t2[:, 1024:2048], C23[:], U,
                            R[:, 1024:2048], Alu.mult, Alu.mult)
                    z2 = chan.tile([128, 2048], fp16, tag="z2", bufs=4)
                    if d in C["Z2_POOL_CHANS"]:
                        nc.gpsimd.tensor_mul(z2[:], t2[:], mIs[:])
                    elif C["Z2_SPLIT"]:
                        nc.gpsimd.tensor_mul(z2[:, 0:1024], t2[:, 0:1024],
                                             mIs[:, 0:1024])
                        nc.vector.tensor_mul(z2[:, 1024:2048],
                                             t2[:, 1024:2048],
                                             mIs[:, 1024:2048])
                    else:
                        nc.vector.tensor_mul(z2[:], t2[:], mIs[:])
                    s["t2"] = t2
                    s["z2"] = z2

                if C["X_FP8"]:
                    emit_blur_A_dr(s["Xd8"], hoff_for(d, "X"), consume_X)
                    if len(pending) > PENDING:
                        flush_C()
                else:
                    blur(s["Xd"], hoff_for(d, "X"), consume_X)
                s["consume_P"] = consume_P

            def emit_P(d):
                s = ch_state[d]
                blur(s["Pd"], hoff_for(d, "P"), s["consume_P"])

            def emit_XP(d):
                emit_XP_head(d)
                emit_P(d)

            def emit_ab_head(d):
                s = ch_state[d]
                if "mp1" in s:   # channels < LEAD: bundle not yet emitted
                    s.pop("mp1")()
                v = chan.tile([128, 2048], bf16, tag="v", bufs=2)
                o = chan.tile([128, 2048], bf16 if C["OUT_BF16"] else f32,
                              tag="o", bufs=2)

                def consume_a(C01, C23):
                    if C23 is None:
                        stt(C["V_ENG"][0], v[:], C01, U, I[:],
                            Alu.mult, Alu.mult)
                    else:
                        stt(C["V_ENG"][0], v[:, 0:1024], C01[:], U,
                            I[:, 0:1024], Alu.mult, Alu.mult)
                        stt(C["V_ENG"][1], v[:, 1024:2048], C23[:],
                            U, I[:, 1024:2048], Alu.mult, Alu.mult)

                def consume_b(C01, C23):
                    if d in C["O_PE_CHANS"] and C23 is not None:
                        # C_b += 121*v on PE, then o = U*C_b on Act
                        for h, Ch in ((0, C01), (1, C23)):
                            lo = 1024 * h
                            nc.tensor.matmul(
                                Ch[:], Idm[:], v[:, lo:lo + 1024],
                                start=False, stop=True,
                                skip_group_check=True)
                            copy_half("A", o[:, lo:lo + 1024], Ch[:],
                                      scale=U)
                            if C["STORE_HALVES"]:
                                nc.sync.dma_start(
                                    out=out_d[d, 256 * h:256 * (h + 1)]
                                    .rearrange("(j p) w -> p j w", p=128),
                                    in_=o[:, lo:lo + 1024].rearrange(
                                        "p (j w) -> p j w", j=2))
                        if not C["STORE_HALVES"]:
                            nc.sync.dma_start(
                                out=out_d[d].rearrange(
                                    "(j p) w -> p j w", p=128),
                                in_=o.rearrange("p (j w) -> p j w", j=4))
                        return
                    if C23 is None:
                        stt(C["O_ENG"][0], o[:], C01, U, v[:],
                            Alu.mult, Alu.add)
                    else:
                        stt(C["O_ENG"][0], o[:, 0:1024], C01[:], U,
                            v[:, 0:1024], Alu.mult, Alu.add)
                        if C["STORE_HALVES"]:
                            nc.sync.dma_start(
                                out=out_d[d, 0:256].rearrange(
                                    "(j p) w -> p j w", p=128),
                                in_=o[:, 0:1024].rearrange(
                                    "p (j w) -> p j w", j=2))
                        stt(C["O_ENG"][1], o[:, 1024:2048], C23[:], U,
                            v[:, 1024:2048], Alu.mult, Alu.add)
                    if C["STORE_HALVES"] and C23 is not None:
                        nc.sync.dma_start(
                            out=out_d[d, 256:512].rearrange(
                                "(j p) w -> p j w", p=128),
                            in_=o[:, 1024:2048].rearrange(
                                "p (j w) -> p j w", j=2))
                    else:
                        nc.sync.dma_start(
                            out=out_d[d].rearrange("(j p) w -> p j w", p=128),
                            in_=o.rearrange("p (j w) -> p j w", j=4))

                blur([(s["t2"], G16), (s["t1"], Gn16)], hoff_for(d, "a"),
                     consume_a)
                s["emit_b"] = lambda: blur(
                    [(s["z1"], G16), (s["z2"], Gn16)], hoff_for(d, "b"),
                    consume_b)

            def emit_b(d):
                s = ch_state[d]
                # deferred mp/t1/z1 upper halves of the X/P-phase channel
                # running LEAD ahead: fills the Act stall at the dual blurs
                dx = d + LEAD
                if dx in ch_state and "mp1" in ch_state[dx]:
                    ch_state[dx].pop("mp1")()
                s["emit_b"]()
                del ch_state[d]

            def emit_ab(d):
                emit_ab_head(d)
                emit_b(d)

            # channel schedule: X/P run ~LEAD channels ahead of a/b
            load_pair(0, singles=C["LOAD0_SINGLES"])
            if C["DEFER_GN"]:
                emit_gn()
            if C["PRE_XP0"]:
                ld(I, img_d[0])
            load_pair(2)
            if LEAD >= 4:
                load_pair(4)
            make_Pd(0)
            make_Pd(1)
            first_d = 0
            if C["XMID"]:
                # I, X0, I2, P0: Act's first op is X0's T1 evac (~5us in)
                blur(I, C["IMG_HOFF"], consume_I)
                emit_XP_head(0)
                blur(I2, C["IMG_HOFF"], consume_I2)
                emit_P(0)
                make_Pd(2)
                dn = LEAD + 1
                if dn < D and dn % 2 == 0:
                    load_pair(dn)
                first_d = 1
            elif C["PRE_XP0"]:
                # X0 blur first so Act gets work ~3us in; img blurs slot
                # behind it, P0 flushes only after I2 (R-chain has no cycle)
                nc.vector.tensor_mul(I2[:], I[:], I[:])
                emit_XP_head(0)
                blur(I, C["IMG_HOFF"], consume_I)
                blur(I2, C["IMG_HOFF"], consume_I2)
                emit_P(0)
                make_Pd(2)
                first_d = 1
            ab_next = [0]
            for d in range(D + LEAD):
                if d < first_d:
                    continue
                if d >= D and ab_next[0] >= D:
                    break
                if d + 2 < D:
                    make_Pd(d + 2)
                if C["INTERLEAVE"]:
                    if d < D:
                        emit_XP_head(d)
                    if d >= LEAD:
                        emit_ab_head(ab_next[0])
                    if d < D:
                        emit_P(d)
                        dn = d + LEAD + 1
                        if dn < D and dn % 2 == 0:
                            load_pair(dn)
                    if d >= LEAD:
                        emit_b(ab_next[0])
                        ab_next[0] += 1
                else:
                    if d < D:
                        emit_XP(d)
                        dn = d + LEAD + 1
                        if dn < D and dn % 2 == 0:
                            load_pair(dn)
                    if d >= LEAD:
                        emit_ab(ab_next[0])
                        ab_next[0] += 1
                if TAPER and D - LEAD <= d < D and ab_next[0] < D:
                    emit_ab(ab_next[0])
                    ab_next[0] += 1
            while pending:
                flush_C()

    nc.compile()
    return nc


_NC_CACHE = None


def kernel(feat: np.ndarray, img: np.ndarray) -> np.ndarray:
    global _NC_CACHE
    from concourse.bass_utils import run_bass_kernel_spmd

    if _NC_CACHE is None:
        _NC_CACHE = _build_bass()
    nc = _NC_CACHE
    g = _g_packed()
    feat16 = np.asarray(feat, np.float32).astype(ml_dtypes.bfloat16)
    img16 = np.asarray(img, np.float32).astype(ml_dtypes.bfloat16)
    gn = np.negative(g)
    idm = (121.0 * np.eye(128, dtype=np.float32)).astype(ml_dtypes.bfloat16)
    feat8 = np.asarray(feat, np.float32).astype(ml_dtypes.float8_e4m3)
    g8 = g.astype(ml_dtypes.float8_e4m3)
    z8 = np.zeros_like(g8)
    g0 = np.concatenate([g8, z8], axis=1)
    g1 = np.concatenate([z8, g8], axis=1)
    in_maps = [
        {"feat": feat16[c], "img": img16[c], "gmat": g, "gneg": gn,
         "idm": idm, "feat8": feat8[c], "g0": g0, "g1": g1}
        for c in range(NCORES)
    ]
    res = run_bass_kernel_spmd(nc, in_maps, list(range(NCORES)))
    out = np.stack([res.results[c]["out"] for c in range(NCORES)], axis=0)
    return np.ascontiguousarray(out.astype(np.float32))
